# revision 8
# baseline (speedup 1.0000x reference)
"""Trainium2 Bass kernel for nn_DiscretizedGaussian (discretized-Gaussian log-likelihood).

Computation per element (mean m, logvar lv, data x):
    idx   = rint(127.5*(x+1))                     (bin index, 0..255; int32-cast rint)
    t'    = idx - 128*m                           (sign-flipped, 128-scaled "x_sel - m")
    iv    = exp(-lv - ln(128))                    (= inv_std/128)
    u+    = (t' + (hh-127.5)) * iv  = -v_minus    (hh = 128/255)
    u-    = (t' - (hh+127.5)) * iv  = -v_plus
    z~    = (u^2 + 1/0.044715) * u    ;  T = tanh(b2 * z~),  b2 = sqrt(2/pi)*0.044715
    d     = T+ - T-  = tanh(P(v_plus)) - tanh(P(v_minus))   (odd symmetry)
    ll    = log(0.5*d + 1e-10)                    (== log(max(cdf_d, 1e-10)) for d>=0)
    out_s = sum over all elements of sample s.

Engine split per [128, 2048] block (24 blocks/core, 8 cores data-parallel over batch),
assignments chosen from HW-measured per-op costs:
    DVE : idx chain (int32-cast rint, 2 ops), u+/u- (scalar_tensor_tensor from PSUM),
          z~ (STT in-place over the squares)
    Pool: squares (tensor_tensor), d = T+ - T- (tensor_tensor)
    ACT : exp, 2x tanh, ln@2048 (+accum_out = free per-partition reduce); blocks
          processed in pairs with all Ln after all exp/tanh to minimize ACT
          table-set switches (exp/tanh and ln live in different table sets)
    PE  : t' = I@idx + (-128I)@m accumulating matmuls (fp32 matmul is 1/4 rate;
          only this stays on PE), plus the final per-sample G-matmul reduce.
"""
import sys
for _p in ("/opt/trn_rl_repo", "/opt/trn_rl_repo/concourse"):
    if _p not in sys.path:
        sys.path.insert(0, _p)

from contextlib import ExitStack
import numpy as np

import concourse.bass as bass  # noqa: F401
import concourse.tile as tile
from concourse import bacc, mybir
from concourse import bass_utils

F32 = mybir.dt.float32
I32 = mybir.dt.int32
P = 128
FB = 2048                 # free-dim block size
NBLK = 24                 # blocks per core
GRP = 2                   # blocks per ACT-table group
FREE = FB * NBLK          # 49152 free elems per partition per core
NCORE = 8
SPB = 8                   # samples per core (64 / 8)
B, C, H, W = 64, 3, 512, 512

LN128 = float(np.log(np.float64(128.0)))
HH = float(np.float64(128.0) / np.float64(255.0))
CP = float(np.float64(HH) - 127.5)
CM = float(-np.float64(HH) - 127.5)
CC = float(np.float64(1.0) / np.float64(0.044715))
B2 = float(np.float64(0.7978845608028654) * np.float64(0.044715))

_CACHE = {}


def _consts_np():
    I = np.eye(P, dtype=np.float32)
    w_m = (-128.0 * I).astype(np.float32)
    G = np.zeros((P, SPB), np.float32)
    for k in range(P):
        G[k, k // 16] = 1.0
    bias_exp = np.full((P, 1), -LN128, np.float32)
    bias_ln = np.full((P, 1), 1e-10, np.float32)
    return np.ascontiguousarray(
        np.concatenate([I, w_m, G, bias_exp, bias_ln], axis=1),
        dtype=np.float32,
    )  # [128, 266]


def _build(reps=1):
    A = mybir.AluOpType
    AF = mybir.ActivationFunctionType
    nc = bacc.Bacc(
        "TRN2",
        target_bir_lowering=False,
        debug=False,
        enable_asserts=False,
        num_devices=NCORE,
    )
    m_in = nc.dram_tensor("m_in", [P, FREE], F32, kind="ExternalInput").ap()
    lv_in = nc.dram_tensor("lv_in", [P, FREE], F32, kind="ExternalInput").ap()
    x_in = nc.dram_tensor("x_in", [P, FREE], F32, kind="ExternalInput").ap()
    c_in = nc.dram_tensor("c_in", [P, 266], F32, kind="ExternalInput").ap()
    o_out = nc.dram_tensor("o_out", [1, SPB], F32, kind="ExternalOutput").ap()

    with tile.TileContext(nc) as tc, ExitStack() as ctx:
        pin = ctx.enter_context(tc.tile_pool(name="pin", bufs=2))
        psc = ctx.enter_context(tc.tile_pool(name="psc", bufs=2))
        piv = ctx.enter_context(tc.tile_pool(name="piv", bufs=2))
        pu = ctx.enter_context(tc.tile_pool(name="pu", bufs=3))
        psq = ctx.enter_context(tc.tile_pool(name="psq", bufs=3))
        pT = ctx.enter_context(tc.tile_pool(name="pT", bufs=4))
        pd = ctx.enter_context(tc.tile_pool(name="pd", bufs=2))
        pone = ctx.enter_context(tc.tile_pool(name="pone", bufs=1))
        pps_t = ctx.enter_context(tc.tile_pool(name="pps_t", bufs=2, space="PSUM"))

        consts = pone.tile([P, 266], F32, tag="consts")
        nc.sync.dma_start(consts[:], c_in[:])
        W_IDX = consts[:, 0:128]
        W_M = consts[:, 128:256]
        G = consts[:, 256:264]
        BIAS_EXP = consts[:, 264:265]
        BIAS_LN = consts[:, 265:266]
        partials = pone.tile([P, NBLK], F32, tag="partials")

        def stage1(b):
            """DMA + idx + exp + t' + u's + squares + z~ + tanh for block b."""
            c0 = b * FB
            x_t = pin.tile([P, FB], F32, tag="x", name=f"x{b}")
            nc.sync.dma_start(x_t[:], x_in[:, c0:c0 + FB])
            m_t = pin.tile([P, FB], F32, tag="m", name=f"m{b}")
            nc.sync.dma_start(m_t[:], m_in[:, c0:c0 + FB])
            lv_t = pin.tile([P, FB], F32, tag="lv", name=f"lv{b}")
            nc.sync.dma_start(lv_t[:], lv_in[:, c0:c0 + FB])

            # idx = rint(127.5*(x+1)) via int32-convert (verified ties-even)
            wi_t = psc.tile([P, FB], I32, tag="wi", name=f"wi{b}")
            nc.vector.tensor_scalar(wi_t[:], x_t[:], 1.0, 127.5, A.add, A.mult)
            idx_t = psc.tile([P, FB], F32, tag="idx", name=f"idx{b}")
            nc.vector.tensor_copy(idx_t[:], wi_t[:])

            iv_t = piv.tile([P, FB], F32, tag="iv", name=f"iv{b}")
            nc.scalar.activation(iv_t[:], lv_t[:], AF.Exp, bias=BIAS_EXP, scale=-1.0)

            # t' = idx - 128*m on PE, one [128, 2048] psum tile (4 banks)
            t_ps = pps_t.tile([P, FB], F32, tag="t", name=f"t{b}")
            for h in range(4):
                ss = slice(h * 512, (h + 1) * 512)
                nc.tensor.matmul(t_ps[:, ss], W_IDX, idx_t[:, ss],
                                 start=True, stop=False)
                nc.tensor.matmul(t_ps[:, ss], W_M, m_t[:, ss],
                                 start=False, stop=True)

            up_t = pu.tile([P, FB], F32, tag="u", name=f"up{b}")
            um_t = pu.tile([P, FB], F32, tag="u", name=f"um{b}")
            nc.vector.scalar_tensor_tensor(up_t[:], t_ps[:], CP, iv_t[:],
                                           A.add, A.mult)
            nc.vector.scalar_tensor_tensor(um_t[:], t_ps[:], CM, iv_t[:],
                                           A.add, A.mult)

            sp_t = psq.tile([P, FB], F32, tag="s", name=f"sp{b}")
            nc.gpsimd.tensor_tensor(sp_t[:], up_t[:], up_t[:], A.mult)
            sm_t = psq.tile([P, FB], F32, tag="s", name=f"sm{b}")
            nc.gpsimd.tensor_tensor(sm_t[:], um_t[:], um_t[:], A.mult)

            # z~ = (s + CC) * u, in place over s
            nc.vector.scalar_tensor_tensor(sp_t[:], sp_t[:], CC, up_t[:],
                                           A.add, A.mult)
            nc.vector.scalar_tensor_tensor(sm_t[:], sm_t[:], CC, um_t[:],
                                           A.add, A.mult)

            Tp_t = pT.tile([P, FB], F32, tag="T", name=f"Tp{b}")
            nc.scalar.activation(Tp_t[:], sp_t[:], AF.Tanh, scale=B2)
            Tm_t = pT.tile([P, FB], F32, tag="T", name=f"Tm{b}")
            nc.scalar.activation(Tm_t[:], sm_t[:], AF.Tanh, scale=B2)
            return Tp_t, Tm_t

        def stage2(b, Tp_t, Tm_t):
            """d = T+ - T- (Pool) and ln+accum (ACT) for block b."""
            d_t = pd.tile([P, FB], F32, tag="d", name=f"d{b}")
            nc.gpsimd.tensor_tensor(d_t[:], Tp_t[:], Tm_t[:], A.subtract)
            # ln output overwrites d in place; only accum_out is consumed
            nc.scalar.activation(d_t[:], d_t[:], AF.Ln,
                                 bias=BIAS_LN, scale=0.5,
                                 accum_out=partials[:, b:b + 1])

        def full_pass(_i=None):
            for g in range(NBLK // GRP):
                Ts = [stage1(g * GRP + i) for i in range(GRP)]
                for i in range(GRP):
                    stage2(g * GRP + i, *Ts[i])

        if reps == 1:
            full_pass()
        else:
            tc.For_i_unrolled(0, reps, 1, full_pass, max_unroll=1)

        part_sum = pone.tile([P, 1], F32, tag="psum1")
        nc.vector.tensor_reduce(part_sum[:], partials[:],
                                axis=mybir.AxisListType.X, op=A.add)
        out_ps = pps_t.tile([1, SPB], F32, tag="t", name="outp")
        nc.tensor.matmul(out_ps[:], part_sum[:], G, start=True, stop=True)
        out_sb = pone.tile([1, SPB], F32, tag="outs")
        nc.vector.tensor_copy(out_sb[:], out_ps[:])
        nc.sync.dma_start(o_out[:], out_sb[:])
    nc.compile()
    return nc


def _get_nc(reps=1):
    key = f"nc{reps}"
    if key not in _CACHE:
        _CACHE[key] = _build(reps)
    return _CACHE[key]


def _make_in_maps(mean, logvar, x):
    consts = _consts_np()
    in_maps = []
    for k in range(NCORE):
        sl = slice(k * SPB, (k + 1) * SPB)
        in_maps.append({
            "m_in": np.ascontiguousarray(mean[sl], dtype=np.float32).reshape(P, FREE),
            "lv_in": np.ascontiguousarray(logvar[sl], dtype=np.float32).reshape(P, FREE),
            "x_in": np.ascontiguousarray(x[sl], dtype=np.float32).reshape(P, FREE),
            "c_in": consts,
        })
    return in_maps


def _run(in_maps, trace=False):
    nc = _get_nc()
    return bass_utils.run_bass_kernel_spmd(
        nc, in_maps, core_ids=list(range(NCORE)), trace=trace)


def kernel(mean, logvar, x):
    assert mean.shape == (B, C, H, W), mean.shape
    res = _run(_make_in_maps(mean, logvar, x), trace=False)
    out = np.concatenate([r["o_out"].reshape(SPB) for r in res.results])
    return out.astype(np.float32)


if __name__ == "__main__":
    rng = np.random.default_rng(0)
    m = (rng.standard_normal((B, C, H, W)) * 0.1).astype(np.float32)
    lv = (rng.standard_normal((B, C, H, W)) * 0.1 - 2.0).astype(np.float32)
    xx = rng.uniform(-1.0, 1.0 - 1e-6, (B, C, H, W)).astype(np.float32)
    out = kernel(m, lv, xx)
    print("kernel out[:8]:", out[:8])
